# revision 22
# baseline (speedup 1.0000x reference)
"""GCNConv message-passing kernel for 8 Trainium2 NeuronCores.

Strategy (1D dst-node partitioning, v2):
  - Host: shard edges by dst across 8 cores (core c owns dst rows
    [c*12500, (c+1)*12500)). Pre-scale node features by rsqrt(out-degree)
    and cast to bf16 (halves gather bytes; the dst-side rsqrt factor is
    applied on device as a per-partition scale during PSUM evacuation).
    Edges sorted by (128-node dst "window", src bank); each (window, bank)
    group packs its edges contiguously into 128-edge chunks, chunk counts
    equalized across cores so ONE SPMD program serves all 8. Pad slots use
    idx=0 (gathers a real row, zeroed by the one-hot S), so every slab
    element is always written and one static num_idxs_reg serves all cores.
  - Device, per super-window (SW windows): one dma_gather per (window,
    bank) group pulls x[src] rows into SBUF ([128 edges] x [128 feat]
    slabs), round-robined over 4 SWDGE queues so descriptor generation
    spreads across all four Q7 core pairs. One bulk DVE tensor_tensor
    (is_equal with broadcast APs) builds all of the super-window's one-hot
    S chunks at once; the PE accumulates S^T @ gathered_x per window into
    PSUM (bf16 matmuls); ACT copies PSUM -> SBUF applying the dst-side
    rsqrt(deg) scale; HWDGE stores the window's output rows.
  - Host: concatenate the 8 output shards.
"""

import os
import sys

sys.path.insert(0, "/opt/trn_rl_repo")

import numpy as np
import ml_dtypes

P = 128  # partitions / window node count / chunk edge count
NCORES = 8
SW = 4  # windows per super-window (slab/pipeline granularity)
BK = 25000  # src rows per gather bank (int16 index reach)
NQ = 4  # SWDGE queues (one per Q7 core pair)

_CACHE = {}
LAST_RESULT = None

BF16 = ml_dtypes.bfloat16


def _plan(x, src, dst):
    """Host-side sharding/sorting. Returns per-core packed device tables and
    the (core-uniform) static layout."""
    n, d = x.shape
    shard = -(-n // NCORES)
    n_win = -(-shard // P)
    b_cnt = -(-n // BK)
    n_grp = n_win * b_cnt
    n_sw = -(-n_win // SW)

    deg = np.bincount(src, minlength=n).astype(np.float32)
    deg = np.maximum(deg, np.float32(1.0))
    rs = (1.0 / np.sqrt(deg)).astype(np.float32)
    xs = (x * rs[:, None]).astype(BF16)

    core_of = dst // shard
    counts = np.zeros((NCORES, n_grp), dtype=np.int64)
    per_core = []
    for c in range(NCORES):
        sel = np.nonzero(core_of == c)[0]
        dloc = (dst[sel] - c * shard).astype(np.int64)
        b = (src[sel] // BK).astype(np.int64)
        key = (dloc >> 7) * b_cnt + b
        order = np.argsort(key, kind="stable")
        sel = sel[order]
        key = key[order]
        counts[c] = np.bincount(key, minlength=n_grp)
        per_core.append(
            (
                (src[sel] - b[order] * BK).astype(np.int16),
                (dloc[order] & 127).astype(np.float32),
                key,
            )
        )

    m_g = (-(-counts.max(axis=0) // P)).reshape(n_win, b_cnt)
    empty_w = m_g.sum(axis=1) == 0
    m_g[empty_w, 0] = 1  # every window needs >=1 chunk to reset PSUM

    # global chunk order: super-window major, then window, then bank, so a
    # window's chunks are contiguous and a super-window's chunks form one
    # contiguous slab range.
    chunk_start = np.zeros((n_win, b_cnt), dtype=np.int64)
    win_range = []  # (first_chunk, end_chunk) global, per window
    sw_base = np.zeros(n_sw, dtype=np.int64)
    sw_chunks = np.zeros(n_sw, dtype=np.int64)
    pos = 0
    for s in range(n_sw):
        sw_base[s] = pos
        for w in range(s * SW, min((s + 1) * SW, n_win)):
            w0 = pos
            for b in range(b_cnt):
                chunk_start[w, b] = pos
                pos += m_g[w, b]
            win_range.append((w0, pos))
        sw_chunks[s] = pos - sw_base[s]
    tc = pos
    CS = int(sw_chunks.max())

    # gather calls: one per (window, bank) group with any data on any core,
    # split at 8 chunks (SWDGE ring safety). Groups empty on every core are
    # skipped entirely. Each core's real indices are topped up with idx=0
    # filler to the cross-core max (so one static num_idxs_reg serves all
    # cores), and the remaining tail holds idx=-1, which the SWDGE ucode
    # trims before descriptor generation: tail padding costs no descriptors
    # and no DMA bytes.
    grp_max = counts.max(axis=0).reshape(n_win, b_cnt)
    calls = []  # (bank, c0, c1, sw, reg)
    for s in range(n_sw):
        for w in range(s * SW, min((s + 1) * SW, n_win)):
            for b in range(b_cnt):
                if grp_max[w, b] == 0:
                    continue
                q0 = int(chunk_start[w, b])
                q1 = q0 + int(m_g[w, b])
                for qq in range(q0, q1, 8):
                    qe = min(qq + 8, q1)
                    calls.append((b, qq, qe, s, (qe - qq) * P))

    # meta int16 layout: [dwin bf16 (tc, padded to even)] [iota bf16 (P)]
    # [dscale f32 (2*n_win)] [idx plane (8*tc)]
    tc2 = tc + (tc & 1)
    off_iota = tc2
    off_dsc = tc2 + P
    off_idx = off_dsc + 2 * n_win
    m16 = off_idx + 8 * tc

    grp_start_flat = chunk_start.reshape(-1)
    iota_bits = (
        np.broadcast_to(np.arange(P, dtype=BF16), (P, P)).copy().view(np.int16)
    )

    tables = []
    for c in range(NCORES):
        idx_rel, dwin, key = per_core[c]
        cum = np.cumsum(counts[c])
        starts = cum - counts[c]
        rank = np.arange(len(key), dtype=np.int64) - starts[key]
        slot = grp_start_flat[key] * P + rank

        dwin_t = np.full((tc, P), -1.0, dtype=BF16)
        dwin_t[slot >> 7, slot & 127] = dwin.astype(BF16)
        dwin_cols = np.zeros((P, tc2), dtype=np.int16)
        dwin_cols[:, :tc] = np.ascontiguousarray(dwin_t.T).view(np.int16)

        idx_flat = np.zeros(tc * P, dtype=np.int16)
        idx_flat[slot] = idx_rel
        s_all = np.arange(tc * P, dtype=np.int64)
        idx_w = np.zeros((16, 8 * tc), dtype=np.int16)
        idx_w[s_all & 15, s_all >> 4] = idx_flat
        idx_full = np.tile(idx_w, (8, 1))

        node = c * shard + np.arange(n_win * P, dtype=np.int64)
        dsc = np.where(node < n, rs[np.minimum(node, n - 1)], np.float32(1.0))
        dsc = np.ascontiguousarray(
            dsc.reshape(n_win, P).T.astype(np.float32)
        ).view(np.int16)  # [P, 2*n_win]

        meta = np.concatenate([dwin_cols, iota_bits, dsc, idx_full], axis=1)
        assert meta.shape == (P, m16)
        tables.append(np.ascontiguousarray(meta))

    layout = dict(
        shard=shard,
        n_win=n_win,
        rows_pad=n_win * P,
        tc=tc,
        tc2=tc2,
        m16=m16,
        off_iota=off_iota,
        off_dsc=off_dsc,
        off_idx=off_idx,
        CS=CS,
        n_sw=n_sw,
        sw_base=sw_base,
        sw_chunks=sw_chunks,
        calls=calls,
        win_range=win_range,
    )
    return layout, tables, xs


def _trace_program(n, d, layout):
    from concourse import bacc, mybir
    import concourse.tile as tile

    f32 = mybir.dt.float32
    bf16 = mybir.dt.bfloat16
    i16 = mybir.dt.int16

    tc = layout["tc"]
    m16 = layout["m16"]
    CS = layout["CS"]
    n_sw = layout["n_sw"]
    n_win = layout["n_win"]
    sw_base = layout["sw_base"]
    sw_chunks = layout["sw_chunks"]
    calls = layout["calls"]
    win_range = layout["win_range"]

    calls_of_sw = [[] for _ in range(n_sw)]
    for b, c0, c1, s, reg in calls:
        calls_of_sw[s].append((b, c0, c1, reg))

    nc = bacc.Bacc(
        None, target_bir_lowering=False, debug=False, num_swdge_queues=NQ
    )
    xs_d = nc.declare_dram_parameter("xs", [n, d], bf16, isOutput=False)
    meta_d = nc.declare_dram_parameter("meta", [P, m16], i16, isOutput=False)
    y_d = nc.declare_dram_parameter(
        "y", [layout["rows_pad"], d], f32, isOutput=True
    )

    qrr = 0  # SWDGE queue round-robin

    with tile.TileContext(nc) as tc_ctx:
        with (
            tc_ctx.tile_pool(name="meta", bufs=1) as mpool,
            tc_ctx.tile_pool(name="gather", bufs=2) as gpool,
            tc_ctx.tile_pool(name="sel", bufs=2) as spool,
            tc_ctx.tile_pool(name="out", bufs=3) as opool,
            tc_ctx.tile_pool(name="acc", bufs=2, space="PSUM") as pspool,
        ):
            meta_sb = mpool.tile([P, m16], i16)
            nc.sync.dma_start(out=meta_sb[:], in_=meta_d[:])
            dwin_sb = meta_sb[:, 0:tc].bitcast(bf16)  # [P, tc]
            iota_sb = meta_sb[
                :, layout["off_iota"] : layout["off_iota"] + P
            ].bitcast(bf16)  # [P, P]
            dsc_sb = meta_sb[
                :, layout["off_dsc"] : layout["off_dsc"] + 2 * n_win
            ].bitcast(f32)  # [P, n_win]
            idx_sb = meta_sb[:, layout["off_idx"] :]  # [P, 8*tc] i16

            for s in range(n_sw):
                base = int(sw_base[s])
                swc = int(sw_chunks[s])
                g = gpool.tile([P, CS, d], bf16, tag="g")
                for b, c0, c1, reg in calls_of_sw[s]:
                    nc.gpsimd.dma_gather(
                        out_ap=g[:, c0 - base : c1 - base, :],
                        in_ap=xs_d[b * BK : min(n, (b + 1) * BK), :],
                        idxs_ap=idx_sb[:, c0 * 8 : c1 * 8],
                        num_idxs=(c1 - c0) * P,
                        num_idxs_reg=reg,
                        elem_size=d,
                        queue_num=qrr,
                    )
                    qrr = (qrr + 1) % NQ

                sel = spool.tile([P, CS, P], bf16, tag="s")
                nc.vector.tensor_tensor(
                    out=sel[:, 0:swc, :],
                    in0=iota_sb.unsqueeze(1).broadcast_to([P, swc, P]),
                    in1=dwin_sb[:, base : base + swc]
                    .unsqueeze(2)
                    .broadcast_to([P, swc, P]),
                    op=mybir.AluOpType.is_equal,
                )

                for w in range(s * SW, min((s + 1) * SW, n_win)):
                    w0, w1 = win_range[w]
                    ps = pspool.tile([P, P], f32, tag="ps")
                    nq = w1 - w0
                    for j in range(nq):
                        qc = w0 - base + j
                        nc.tensor.matmul(
                            out=ps[:],
                            lhsT=sel[:, qc, :],
                            rhs=g[:, qc, :],
                            start=(j == 0),
                            stop=(j == nq - 1),
                        )
                    o = opool.tile([P, P], f32, tag="o")
                    nc.scalar.activation(
                        out=o[:],
                        in_=ps[:],
                        func=mybir.ActivationFunctionType.Copy,
                        scale=dsc_sb[:, w : w + 1],
                    )
                    nc.sync.dma_start(
                        out=y_d[w * P : (w + 1) * P, :], in_=o[:]
                    )

    return nc


def _build_program(n, d, layout):
    nc = _trace_program(n, d, layout)
    nc.compile()
    return nc


def kernel(x, src, dst):
    x = np.ascontiguousarray(np.asarray(x, dtype=np.float32))
    src = np.asarray(src).astype(np.int64)
    dst = np.asarray(dst).astype(np.int64)
    n, d = x.shape

    layout, tables, xs = _plan(x, src, dst)

    key = (n, d, layout["tc"], tuple(layout["calls"]), tuple(layout["win_range"]))
    if key not in _CACHE:
        _CACHE[key] = _build_program(n, d, layout)
    nc = _CACHE[key]

    from concourse.bass_utils import run_bass_kernel_spmd

    in_maps = [{"xs": xs, "meta": tables[c]} for c in range(NCORES)]
    trace = os.environ.get("KERNEL_TRACE", "0") not in ("", "0")
    kw = {}
    if trace:
        kw["trace"] = True
        td = os.environ.get("KERNEL_TRACE_DIR")
        if td:
            kw["tmpdir"] = td
    res = run_bass_kernel_spmd(nc, in_maps, list(range(NCORES)), **kw)
    global LAST_RESULT
    LAST_RESULT = res

    shard = layout["shard"]
    out = np.empty((n, d), dtype=np.float32)
    for c in range(NCORES):
        lo = c * shard
        hi = min(n, lo + shard)
        out[lo:hi] = res.results[c]["y"][: hi - lo]
    return out


# revision 28
# speedup vs baseline: 1.0612x; 1.0612x over previous
"""GCNConv message-passing kernel for 8 Trainium2 NeuronCores.

Strategy (1D dst-node partitioning, v2):
  - Host: shard edges by dst across 8 cores (core c owns dst rows
    [c*12500, (c+1)*12500)). Pre-scale node features by rsqrt(out-degree)
    and cast to bf16 (halves gather bytes; the dst-side rsqrt factor is
    applied on device as a per-partition scale during PSUM evacuation).
    Edges sorted by (128-node dst "window", src bank); each (window, bank)
    group packs its edges contiguously into 128-edge chunks, chunk counts
    equalized across cores so ONE SPMD program serves all 8. Pad slots use
    idx=0 (gathers a real row, zeroed by the one-hot S), so every slab
    element is always written and one static num_idxs_reg serves all cores.
  - Device, per super-window (SW windows): one dma_gather per (window,
    bank) group pulls x[src] rows into SBUF ([128 edges] x [128 feat]
    slabs), round-robined over 4 SWDGE queues so descriptor generation
    spreads across all four Q7 core pairs. One bulk DVE tensor_tensor
    (is_equal with broadcast APs) builds all of the super-window's one-hot
    S chunks at once; the PE accumulates S^T @ gathered_x per window into
    PSUM (bf16 matmuls); ACT copies PSUM -> SBUF applying the dst-side
    rsqrt(deg) scale; HWDGE stores the window's output rows.
  - Host: concatenate the 8 output shards.
"""

import os
import sys

sys.path.insert(0, "/opt/trn_rl_repo")

import numpy as np
import ml_dtypes

P = 128  # partitions / window node count / chunk edge count
NCORES = 8
SW = 4  # windows per super-window (slab/pipeline granularity)
BK = 25000  # src rows per gather bank (int16 index reach)
NQ = 4  # SWDGE queues (one per Q7 core pair)

_CACHE = {}
LAST_RESULT = None

BF16 = ml_dtypes.bfloat16


def _plan(x, src, dst):
    """Host-side sharding/sorting. Returns per-core packed device tables and
    the (core-uniform) static layout."""
    n, d = x.shape
    shard = -(-n // NCORES)
    n_win = -(-shard // P)
    b_cnt = -(-n // BK)
    n_grp = n_win * b_cnt
    n_sw = -(-n_win // SW)

    deg = np.bincount(src, minlength=n).astype(np.float32)
    deg = np.maximum(deg, np.float32(1.0))
    rs = (1.0 / np.sqrt(deg)).astype(np.float32)
    xs = (x * rs[:, None]).astype(BF16)

    core_of = dst // shard
    counts = np.zeros((NCORES, n_grp), dtype=np.int64)
    per_core = []
    for c in range(NCORES):
        sel = np.nonzero(core_of == c)[0]
        dloc = (dst[sel] - c * shard).astype(np.int64)
        b = (src[sel] // BK).astype(np.int64)
        key = (dloc >> 7) * b_cnt + b
        order = np.argsort(key, kind="stable")
        sel = sel[order]
        key = key[order]
        counts[c] = np.bincount(key, minlength=n_grp)
        per_core.append(
            (
                (src[sel] - b[order] * BK).astype(np.int16),
                (dloc[order] & 127).astype(np.float32),
                key,
            )
        )

    m_g = (-(-counts.max(axis=0) // P)).reshape(n_win, b_cnt)
    empty_w = m_g.sum(axis=1) == 0
    m_g[empty_w, 0] = 1  # every window needs >=1 chunk to reset PSUM

    # global chunk order: super-window major, then BANK, then window, so each
    # (super-window, bank) run is one contiguous slab range servable by a
    # single big dma_gather call (amortizing the per-call serial decode cost
    # on the Pool sequencer), while a super-window's chunks stay contiguous
    # for the bulk S build. A window's chunks are then up to 4 disjoint runs.
    chunk_start = np.zeros((n_win, b_cnt), dtype=np.int64)
    win_cols = [[] for _ in range(n_win)]
    sw_base = np.zeros(n_sw, dtype=np.int64)
    sw_chunks = np.zeros(n_sw, dtype=np.int64)
    pos = 0
    bank_runs = []  # (bank, c0, c1, sw)
    for s in range(n_sw):
        sw_base[s] = pos
        for b in range(b_cnt):
            r0 = pos
            for w in range(s * SW, min((s + 1) * SW, n_win)):
                chunk_start[w, b] = pos
                win_cols[w].extend(range(pos, pos + m_g[w, b]))
                pos += m_g[w, b]
            if pos > r0:
                bank_runs.append((b, r0, pos, s))
        sw_chunks[s] = pos - sw_base[s]
    tc = pos
    CS = int(sw_chunks.max())

    # gather calls: one per (super-window, bank) run, split at MAXC chunks
    # (1024 idxs — the SWDGE gather cap; bigger calls fault the ring). Pad
    # slots carry idx=0 (a real row, zeroed by the one-hot S) so every slab
    # element is always written and one static num_idxs_reg serves all
    # cores.
    MAXC = 8
    calls = []  # (bank, c0, c1, sw, reg)
    for b, r0, r1, s in bank_runs:
        for qq in range(r0, r1, MAXC):
            qe = min(qq + MAXC, r1)
            calls.append((b, qq, qe, s, (qe - qq) * P))

    # meta int16 layout: [dwin bf16 (tc, padded to even)] [iota bf16 (P)]
    # [dscale f32 (2*n_win)] [idx plane (8*tc)]
    tc2 = tc + (tc & 1)
    off_iota = tc2
    off_dsc = tc2 + P
    off_idx = off_dsc + 2 * n_win
    m16 = off_idx + 8 * tc

    grp_start_flat = chunk_start.reshape(-1)
    iota_bits = (
        np.broadcast_to(np.arange(P, dtype=BF16), (P, P)).copy().view(np.int16)
    )

    tables = []
    for c in range(NCORES):
        idx_rel, dwin, key = per_core[c]
        cum = np.cumsum(counts[c])
        starts = cum - counts[c]
        rank = np.arange(len(key), dtype=np.int64) - starts[key]
        slot = grp_start_flat[key] * P + rank

        dwin_t = np.full((tc, P), -1.0, dtype=BF16)
        dwin_t[slot >> 7, slot & 127] = dwin.astype(BF16)
        dwin_cols = np.zeros((P, tc2), dtype=np.int16)
        dwin_cols[:, :tc] = np.ascontiguousarray(dwin_t.T).view(np.int16)

        idx_flat = np.zeros(tc * P, dtype=np.int16)
        idx_flat[slot] = idx_rel
        s_all = np.arange(tc * P, dtype=np.int64)
        idx_w = np.zeros((16, 8 * tc), dtype=np.int16)
        idx_w[s_all & 15, s_all >> 4] = idx_flat
        idx_full = np.tile(idx_w, (8, 1))

        node = c * shard + np.arange(n_win * P, dtype=np.int64)
        dsc = np.where(node < n, rs[np.minimum(node, n - 1)], np.float32(1.0))
        dsc = np.ascontiguousarray(
            dsc.reshape(n_win, P).T.astype(np.float32)
        ).view(np.int16)  # [P, 2*n_win]

        meta = np.concatenate([dwin_cols, iota_bits, dsc, idx_full], axis=1)
        assert meta.shape == (P, m16)
        tables.append(np.ascontiguousarray(meta))

    layout = dict(
        shard=shard,
        n_win=n_win,
        rows_pad=n_win * P,
        tc=tc,
        tc2=tc2,
        m16=m16,
        off_iota=off_iota,
        off_dsc=off_dsc,
        off_idx=off_idx,
        CS=CS,
        n_sw=n_sw,
        sw_base=sw_base,
        sw_chunks=sw_chunks,
        calls=calls,
        win_cols=win_cols,
    )
    return layout, tables, xs


def _trace_program(n, d, layout):
    from concourse import bacc, mybir
    import concourse.tile as tile

    f32 = mybir.dt.float32
    bf16 = mybir.dt.bfloat16
    i16 = mybir.dt.int16

    tc = layout["tc"]
    m16 = layout["m16"]
    CS = layout["CS"]
    n_sw = layout["n_sw"]
    n_win = layout["n_win"]
    sw_base = layout["sw_base"]
    sw_chunks = layout["sw_chunks"]
    calls = layout["calls"]
    win_cols = layout["win_cols"]

    calls_of_sw = [[] for _ in range(n_sw)]
    for b, c0, c1, s, reg in calls:
        calls_of_sw[s].append((b, c0, c1, reg))

    nc = bacc.Bacc(
        None, target_bir_lowering=False, debug=False, num_swdge_queues=NQ
    )
    xs_d = nc.declare_dram_parameter("xs", [n, d], bf16, isOutput=False)
    meta_d = nc.declare_dram_parameter("meta", [P, m16], i16, isOutput=False)
    y_d = nc.declare_dram_parameter(
        "y", [layout["rows_pad"], d], f32, isOutput=True
    )

    qrr = 0  # SWDGE queue round-robin

    with tile.TileContext(nc) as tc_ctx:
        with (
            tc_ctx.tile_pool(name="meta", bufs=1) as mpool,
            tc_ctx.tile_pool(name="gather", bufs=2) as gpool,
            tc_ctx.tile_pool(name="sel", bufs=2) as spool,
            tc_ctx.tile_pool(name="out", bufs=3) as opool,
            tc_ctx.tile_pool(name="acc", bufs=2, space="PSUM") as pspool,
        ):
            meta_sb = mpool.tile([P, m16], i16)
            nc.sync.dma_start(out=meta_sb[:], in_=meta_d[:])
            dwin_sb = meta_sb[:, 0:tc].bitcast(bf16)  # [P, tc]
            iota_sb = meta_sb[
                :, layout["off_iota"] : layout["off_iota"] + P
            ].bitcast(bf16)  # [P, P]
            dsc_sb = meta_sb[
                :, layout["off_dsc"] : layout["off_dsc"] + 2 * n_win
            ].bitcast(f32)  # [P, n_win]
            idx_sb = meta_sb[:, layout["off_idx"] :]  # [P, 8*tc] i16

            for s in range(n_sw):
                base = int(sw_base[s])
                swc = int(sw_chunks[s])
                g = gpool.tile([P, CS, d], bf16, tag="g")
                for b, c0, c1, reg in calls_of_sw[s]:
                    nc.gpsimd.dma_gather(
                        out_ap=g[:, c0 - base : c1 - base, :],
                        in_ap=xs_d[b * BK : min(n, (b + 1) * BK), :],
                        idxs_ap=idx_sb[:, c0 * 8 : c1 * 8],
                        num_idxs=(c1 - c0) * P,
                        num_idxs_reg=reg,
                        elem_size=d,
                        queue_num=qrr,
                    )
                    qrr = (qrr + 1) % NQ

                sel = spool.tile([P, CS, P], bf16, tag="s")
                nc.vector.tensor_tensor(
                    out=sel[:, 0:swc, :],
                    in0=iota_sb.unsqueeze(1).broadcast_to([P, swc, P]),
                    in1=dwin_sb[:, base : base + swc]
                    .unsqueeze(2)
                    .broadcast_to([P, swc, P]),
                    op=mybir.AluOpType.is_equal,
                )

                for w in range(s * SW, min((s + 1) * SW, n_win)):
                    cols = win_cols[w]
                    ps = pspool.tile([P, P], f32, tag="ps")
                    nq = len(cols)
                    for j in range(nq):
                        qc = cols[j] - base
                        nc.tensor.matmul(
                            out=ps[:],
                            lhsT=sel[:, qc, :],
                            rhs=g[:, qc, :],
                            start=(j == 0),
                            stop=(j == nq - 1),
                        )
                    o = opool.tile([P, P], f32, tag="o")
                    nc.scalar.activation(
                        out=o[:],
                        in_=ps[:],
                        func=mybir.ActivationFunctionType.Copy,
                        scale=dsc_sb[:, w : w + 1],
                    )
                    nc.sync.dma_start(
                        out=y_d[w * P : (w + 1) * P, :], in_=o[:]
                    )

    return nc


def _build_program(n, d, layout):
    nc = _trace_program(n, d, layout)
    nc.compile()
    return nc


def kernel(x, src, dst):
    x = np.ascontiguousarray(np.asarray(x, dtype=np.float32))
    src = np.asarray(src).astype(np.int64)
    dst = np.asarray(dst).astype(np.int64)
    n, d = x.shape

    layout, tables, xs = _plan(x, src, dst)

    key = (n, d, layout["tc"], tuple(layout["calls"]),
           tuple(tuple(q) for q in layout["win_cols"]))
    if key not in _CACHE:
        _CACHE[key] = _build_program(n, d, layout)
    nc = _CACHE[key]

    from concourse.bass_utils import run_bass_kernel_spmd

    in_maps = [{"xs": xs, "meta": tables[c]} for c in range(NCORES)]
    trace = os.environ.get("KERNEL_TRACE", "0") not in ("", "0")
    kw = {}
    if trace:
        kw["trace"] = True
        td = os.environ.get("KERNEL_TRACE_DIR")
        if td:
            kw["tmpdir"] = td
    res = run_bass_kernel_spmd(nc, in_maps, list(range(NCORES)), **kw)
    global LAST_RESULT
    LAST_RESULT = res

    shard = layout["shard"]
    out = np.empty((n, d), dtype=np.float32)
    for c in range(NCORES):
        lo = c * shard
        hi = min(n, lo + shard)
        out[lo:hi] = res.results[c]["y"][: hi - lo]
    return out


# revision 29
# speedup vs baseline: 1.0914x; 1.0284x over previous
"""GCNConv message-passing kernel for 8 Trainium2 NeuronCores.

Strategy (1D dst-node partitioning, v2):
  - Host: shard edges by dst across 8 cores (core c owns dst rows
    [c*12500, (c+1)*12500)). Pre-scale node features by rsqrt(out-degree)
    and cast to bf16 (halves gather bytes; the dst-side rsqrt factor is
    applied on device as a per-partition scale during PSUM evacuation).
    Edges sorted by (128-node dst "window", src bank); each (window, bank)
    group packs its edges contiguously into 128-edge chunks, chunk counts
    equalized across cores so ONE SPMD program serves all 8. Pad slots use
    idx=0 (gathers a real row, zeroed by the one-hot S), so every slab
    element is always written and one static num_idxs_reg serves all cores.
  - Device, per super-window (SW windows): one dma_gather per (window,
    bank) group pulls x[src] rows into SBUF ([128 edges] x [128 feat]
    slabs), round-robined over 4 SWDGE queues so descriptor generation
    spreads across all four Q7 core pairs. One bulk DVE tensor_tensor
    (is_equal with broadcast APs) builds all of the super-window's one-hot
    S chunks at once; the PE accumulates S^T @ gathered_x per window into
    PSUM (bf16 matmuls); ACT copies PSUM -> SBUF applying the dst-side
    rsqrt(deg) scale; HWDGE stores the window's output rows.
  - Host: concatenate the 8 output shards.
"""

import os
import sys

sys.path.insert(0, "/opt/trn_rl_repo")

import numpy as np
import ml_dtypes

P = 128  # partitions / window node count / chunk edge count
NCORES = 8
SW = 4  # windows per super-window (slab/pipeline granularity)
BK = 25000  # src rows per gather bank (int16 index reach)
NQ = 4  # SWDGE queues (one per Q7 core pair)

_CACHE = {}
LAST_RESULT = None

BF16 = ml_dtypes.bfloat16


def _plan(x, src, dst):
    """Host-side sharding/sorting. Returns per-core packed device tables and
    the (core-uniform) static layout."""
    n, d = x.shape
    shard = -(-n // NCORES)
    n_win = -(-shard // P)
    b_cnt = -(-n // BK)
    n_grp = n_win * b_cnt
    n_sw = -(-n_win // SW)

    deg = np.bincount(src, minlength=n).astype(np.float32)
    deg = np.maximum(deg, np.float32(1.0))
    rs = (1.0 / np.sqrt(deg)).astype(np.float32)
    xs = (x * rs[:, None]).astype(BF16)

    core_of = dst // shard
    counts = np.zeros((NCORES, n_grp), dtype=np.int64)
    per_core = []
    for c in range(NCORES):
        sel = np.nonzero(core_of == c)[0]
        dloc = (dst[sel] - c * shard).astype(np.int64)
        b = (src[sel] // BK).astype(np.int64)
        key = (dloc >> 7) * b_cnt + b
        order = np.argsort(key, kind="stable")
        sel = sel[order]
        key = key[order]
        counts[c] = np.bincount(key, minlength=n_grp)
        per_core.append(
            (
                (src[sel] - b[order] * BK).astype(np.int16),
                (dloc[order] & 127).astype(np.float32),
                key,
            )
        )

    m_g = (-(-counts.max(axis=0) // P)).reshape(n_win, b_cnt)
    empty_w = m_g.sum(axis=1) == 0
    m_g[empty_w, 0] = 1  # every window needs >=1 chunk to reset PSUM

    # global chunk order: super-window major, then window, then bank, so a
    # window's chunks are contiguous and a super-window's chunks form one
    # contiguous slab range.
    chunk_start = np.zeros((n_win, b_cnt), dtype=np.int64)
    win_cols = [[] for _ in range(n_win)]
    sw_base = np.zeros(n_sw, dtype=np.int64)
    sw_chunks = np.zeros(n_sw, dtype=np.int64)
    pos = 0
    for s in range(n_sw):
        sw_base[s] = pos
        for w in range(s * SW, min((s + 1) * SW, n_win)):
            for b in range(b_cnt):
                chunk_start[w, b] = pos
                win_cols[w].extend(range(pos, pos + m_g[w, b]))
                pos += m_g[w, b]
        sw_chunks[s] = pos - sw_base[s]
    tc = pos
    CS = int(sw_chunks.max())

    # gather calls: one per (window, bank) group with any data on any core,
    # split at 8 chunks (1024 idxs, the SWDGE gather cap). Pad slots carry
    # idx=0 (a real row, zeroed by the one-hot S) so every slab element is
    # always written and one static num_idxs_reg serves all cores.
    grp_max = counts.max(axis=0).reshape(n_win, b_cnt)
    calls = []  # (bank, c0, c1, sw, reg)
    for s in range(n_sw):
        for w in range(s * SW, min((s + 1) * SW, n_win)):
            for b in range(b_cnt):
                if m_g[w, b] == 0:
                    continue
                q0 = int(chunk_start[w, b])
                q1 = q0 + int(m_g[w, b])
                for qq in range(q0, q1, 8):
                    qe = min(qq + 8, q1)
                    calls.append((b, qq, qe, s, (qe - qq) * P))

    # meta int16 layout: [dwin bf16 (tc, padded to even)] [iota bf16 (P)]
    # [dscale f32 (2*n_win)] [idx plane (8*tc)]
    tc2 = tc + (tc & 1)
    off_iota = tc2
    off_dsc = tc2 + P
    off_idx = off_dsc + 2 * n_win
    m16 = off_idx + 8 * tc

    grp_start_flat = chunk_start.reshape(-1)
    iota_bits = (
        np.broadcast_to(np.arange(P, dtype=BF16), (P, P)).copy().view(np.int16)
    )

    tables = []
    for c in range(NCORES):
        idx_rel, dwin, key = per_core[c]
        cum = np.cumsum(counts[c])
        starts = cum - counts[c]
        rank = np.arange(len(key), dtype=np.int64) - starts[key]
        slot = grp_start_flat[key] * P + rank

        dwin_t = np.full((tc, P), -1.0, dtype=BF16)
        dwin_t[slot >> 7, slot & 127] = dwin.astype(BF16)
        dwin_cols = np.zeros((P, tc2), dtype=np.int16)
        dwin_cols[:, :tc] = np.ascontiguousarray(dwin_t.T).view(np.int16)

        idx_flat = np.zeros(tc * P, dtype=np.int16)
        idx_flat[slot] = idx_rel
        s_all = np.arange(tc * P, dtype=np.int64)
        idx_w = np.zeros((16, 8 * tc), dtype=np.int16)
        idx_w[s_all & 15, s_all >> 4] = idx_flat
        idx_full = np.tile(idx_w, (8, 1))

        node = c * shard + np.arange(n_win * P, dtype=np.int64)
        dsc = np.where(node < n, rs[np.minimum(node, n - 1)], np.float32(1.0))
        dsc = np.ascontiguousarray(
            dsc.reshape(n_win, P).T.astype(np.float32)
        ).view(np.int16)  # [P, 2*n_win]

        meta = np.concatenate([dwin_cols, iota_bits, dsc, idx_full], axis=1)
        assert meta.shape == (P, m16)
        tables.append(np.ascontiguousarray(meta))

    layout = dict(
        shard=shard,
        n_win=n_win,
        rows_pad=n_win * P,
        tc=tc,
        tc2=tc2,
        m16=m16,
        off_iota=off_iota,
        off_dsc=off_dsc,
        off_idx=off_idx,
        CS=CS,
        n_sw=n_sw,
        sw_base=sw_base,
        sw_chunks=sw_chunks,
        calls=calls,
        win_cols=win_cols,
    )
    return layout, tables, xs


def _trace_program(n, d, layout):
    from concourse import bacc, mybir
    import concourse.tile as tile

    f32 = mybir.dt.float32
    bf16 = mybir.dt.bfloat16
    i16 = mybir.dt.int16

    tc = layout["tc"]
    m16 = layout["m16"]
    CS = layout["CS"]
    n_sw = layout["n_sw"]
    n_win = layout["n_win"]
    sw_base = layout["sw_base"]
    sw_chunks = layout["sw_chunks"]
    calls = layout["calls"]
    win_cols = layout["win_cols"]

    calls_of_sw = [[] for _ in range(n_sw)]
    for b, c0, c1, s, reg in calls:
        calls_of_sw[s].append((b, c0, c1, reg))

    nc = bacc.Bacc(
        None, target_bir_lowering=False, debug=False, num_swdge_queues=NQ
    )
    xs_d = nc.declare_dram_parameter("xs", [n, d], bf16, isOutput=False)
    meta_d = nc.declare_dram_parameter("meta", [P, m16], i16, isOutput=False)
    y_d = nc.declare_dram_parameter(
        "y", [layout["rows_pad"], d], f32, isOutput=True
    )

    qrr = 0  # SWDGE queue round-robin

    with tile.TileContext(nc) as tc_ctx:
        with (
            tc_ctx.tile_pool(name="meta", bufs=1) as mpool,
            tc_ctx.tile_pool(name="gather", bufs=2) as gpool,
            tc_ctx.tile_pool(name="sel", bufs=2) as spool,
            tc_ctx.tile_pool(name="out", bufs=3) as opool,
            tc_ctx.tile_pool(name="acc", bufs=2, space="PSUM") as pspool,
        ):
            meta_sb = mpool.tile([P, m16], i16)
            nc.sync.dma_start(out=meta_sb[:], in_=meta_d[:])
            dwin_sb = meta_sb[:, 0:tc].bitcast(bf16)  # [P, tc]
            iota_sb = meta_sb[
                :, layout["off_iota"] : layout["off_iota"] + P
            ].bitcast(bf16)  # [P, P]
            dsc_sb = meta_sb[
                :, layout["off_dsc"] : layout["off_dsc"] + 2 * n_win
            ].bitcast(f32)  # [P, n_win]
            idx_sb = meta_sb[:, layout["off_idx"] :]  # [P, 8*tc] i16

            for s in range(n_sw):
                base = int(sw_base[s])
                swc = int(sw_chunks[s])
                g = gpool.tile([P, CS, d], bf16, tag="g")
                for b, c0, c1, reg in calls_of_sw[s]:
                    nc.gpsimd.dma_gather(
                        out_ap=g[:, c0 - base : c1 - base, :],
                        in_ap=xs_d[b * BK : min(n, (b + 1) * BK), :],
                        idxs_ap=idx_sb[:, c0 * 8 : c1 * 8],
                        num_idxs=(c1 - c0) * P,
                        num_idxs_reg=reg,
                        elem_size=d,
                        queue_num=qrr,
                    )
                    qrr = (qrr + 1) % NQ

                sel = spool.tile([P, CS, P], bf16, tag="s")
                nc.vector.tensor_tensor(
                    out=sel[:, 0:swc, :],
                    in0=iota_sb.unsqueeze(1).broadcast_to([P, swc, P]),
                    in1=dwin_sb[:, base : base + swc]
                    .unsqueeze(2)
                    .broadcast_to([P, swc, P]),
                    op=mybir.AluOpType.is_equal,
                )

                for w in range(s * SW, min((s + 1) * SW, n_win)):
                    cols = win_cols[w]
                    ps = pspool.tile([P, P], f32, tag="ps")
                    nq = len(cols)
                    for j in range(nq):
                        qc = cols[j] - base
                        nc.tensor.matmul(
                            out=ps[:],
                            lhsT=sel[:, qc, :],
                            rhs=g[:, qc, :],
                            start=(j == 0),
                            stop=(j == nq - 1),
                        )
                    o = opool.tile([P, P], f32, tag="o")
                    nc.scalar.activation(
                        out=o[:],
                        in_=ps[:],
                        func=mybir.ActivationFunctionType.Copy,
                        scale=dsc_sb[:, w : w + 1],
                    )
                    nc.sync.dma_start(
                        out=y_d[w * P : (w + 1) * P, :], in_=o[:]
                    )

    return nc


def _build_program(n, d, layout):
    nc = _trace_program(n, d, layout)
    nc.compile()
    return nc


def kernel(x, src, dst):
    x = np.ascontiguousarray(np.asarray(x, dtype=np.float32))
    src = np.asarray(src).astype(np.int64)
    dst = np.asarray(dst).astype(np.int64)
    n, d = x.shape

    layout, tables, xs = _plan(x, src, dst)

    key = (n, d, layout["tc"], tuple(layout["calls"]),
           tuple(tuple(q) for q in layout["win_cols"]))
    if key not in _CACHE:
        _CACHE[key] = _build_program(n, d, layout)
    nc = _CACHE[key]

    from concourse.bass_utils import run_bass_kernel_spmd

    in_maps = [{"xs": xs, "meta": tables[c]} for c in range(NCORES)]
    trace = os.environ.get("KERNEL_TRACE", "0") not in ("", "0")
    kw = {}
    if trace:
        kw["trace"] = True
        td = os.environ.get("KERNEL_TRACE_DIR")
        if td:
            kw["tmpdir"] = td
    res = run_bass_kernel_spmd(nc, in_maps, list(range(NCORES)), **kw)
    global LAST_RESULT
    LAST_RESULT = res

    shard = layout["shard"]
    out = np.empty((n, d), dtype=np.float32)
    for c in range(NCORES):
        lo = c * shard
        hi = min(n, lo + shard)
        out[lo:hi] = res.results[c]["y"][: hi - lo]
    return out


# revision 30
# speedup vs baseline: 1.1674x; 1.0696x over previous
"""GCNConv message-passing kernel for 8 Trainium2 NeuronCores.

Strategy (1D dst-node partitioning, v2):
  - Host: shard edges by dst across 8 cores (core c owns dst rows
    [c*12500, (c+1)*12500)). Pre-scale node features by rsqrt(out-degree)
    and cast to bf16 (halves gather bytes; the dst-side rsqrt factor is
    applied on device as a per-partition scale during PSUM evacuation).
    Edges sorted by (128-node dst "window", src bank); each (window, bank)
    group packs its edges contiguously into 128-edge chunks, chunk counts
    equalized across cores so ONE SPMD program serves all 8. Pad slots use
    idx=0 (gathers a real row, zeroed by the one-hot S), so every slab
    element is always written and one static num_idxs_reg serves all cores.
  - Device, per super-window (SW windows): one dma_gather per (window,
    bank) group pulls x[src] rows into SBUF ([128 edges] x [128 feat]
    slabs), round-robined over 4 SWDGE queues so descriptor generation
    spreads across all four Q7 core pairs. One bulk DVE tensor_tensor
    (is_equal with broadcast APs) builds all of the super-window's one-hot
    S chunks at once; the PE accumulates S^T @ gathered_x per window into
    PSUM (bf16 matmuls); ACT copies PSUM -> SBUF applying the dst-side
    rsqrt(deg) scale; HWDGE stores the window's output rows.
  - Host: concatenate the 8 output shards.
"""

import os
import sys

sys.path.insert(0, "/opt/trn_rl_repo")

import numpy as np
import ml_dtypes

P = 128  # partitions / window node count / chunk edge count
NCORES = 8
SW = 4  # windows per super-window (slab/pipeline granularity)
BK = 25000  # src rows per gather bank (int16 index reach)
NQ = 4  # SWDGE queues (one per Q7 core pair)

_CACHE = {}
LAST_RESULT = None

BF16 = ml_dtypes.bfloat16


def _plan(x, src, dst):
    """Host-side sharding/sorting. Returns per-core packed device tables and
    the (core-uniform) static layout."""
    n, d = x.shape
    shard = -(-n // NCORES)
    n_win = -(-shard // P)
    b_cnt = -(-n // BK)
    n_grp = n_win * b_cnt
    n_sw = -(-n_win // SW)

    deg = np.bincount(src, minlength=n).astype(np.float32)
    deg = np.maximum(deg, np.float32(1.0))
    rs = (1.0 / np.sqrt(deg)).astype(np.float32)
    xs = (x * rs[:, None]).astype(BF16)

    core_of = dst // shard
    counts = np.zeros((NCORES, n_grp), dtype=np.int64)
    per_core = []
    for c in range(NCORES):
        sel = np.nonzero(core_of == c)[0]
        dloc = (dst[sel] - c * shard).astype(np.int64)
        b = (src[sel] // BK).astype(np.int64)
        key = (dloc >> 7) * b_cnt + b
        order = np.argsort(key, kind="stable")
        sel = sel[order]
        key = key[order]
        counts[c] = np.bincount(key, minlength=n_grp)
        per_core.append(
            (
                (src[sel] - b[order] * BK).astype(np.int16),
                (dloc[order] & 127).astype(np.float32),
                key,
            )
        )

    m_g = (-(-counts.max(axis=0) // P)).reshape(n_win, b_cnt)
    empty_w = m_g.sum(axis=1) == 0
    m_g[empty_w, 0] = 1  # every window needs >=1 chunk to reset PSUM

    # global chunk order: super-window major, then window, then bank, so a
    # window's chunks are contiguous and a super-window's chunks form one
    # contiguous slab range.
    chunk_start = np.zeros((n_win, b_cnt), dtype=np.int64)
    win_cols = [[] for _ in range(n_win)]
    sw_base = np.zeros(n_sw, dtype=np.int64)
    sw_chunks = np.zeros(n_sw, dtype=np.int64)
    pos = 0
    for s in range(n_sw):
        sw_base[s] = pos
        for w in range(s * SW, min((s + 1) * SW, n_win)):
            for b in range(b_cnt):
                chunk_start[w, b] = pos
                win_cols[w].extend(range(pos, pos + m_g[w, b]))
                pos += m_g[w, b]
        sw_chunks[s] = pos - sw_base[s]
    tc = pos
    CS = int(sw_chunks.max())

    # gather calls: one per (window, bank) group with any data on any core,
    # split at 8 chunks (1024 idxs, the SWDGE gather cap). Pad slots carry
    # idx=0 (a real row, zeroed by the one-hot S) so every slab element is
    # always written and one static num_idxs_reg serves all cores.
    grp_max = counts.max(axis=0).reshape(n_win, b_cnt)
    calls = []  # (bank, c0, c1, sw, reg)
    for s in range(n_sw):
        for w in range(s * SW, min((s + 1) * SW, n_win)):
            for b in range(b_cnt):
                if m_g[w, b] == 0:
                    continue
                q0 = int(chunk_start[w, b])
                q1 = q0 + int(m_g[w, b])
                for qq in range(q0, q1, 8):
                    qe = min(qq + 8, q1)
                    calls.append((b, qq, qe, s, (qe - qq) * P))

    # meta int16 layout: [dwin bf16 (tc, padded to even)] [iota bf16 (P)]
    # [dscale f32 (2*n_win)] [idx plane (8*tc)]
    tc2 = tc + (tc & 1)
    off_iota = tc2
    off_dsc = tc2 + P
    off_idx = off_dsc + 2 * n_win
    m16 = off_idx + 8 * tc

    grp_start_flat = chunk_start.reshape(-1)
    iota_bits = (
        np.broadcast_to(np.arange(P, dtype=BF16), (P, P)).copy().view(np.int16)
    )

    tables = []
    for c in range(NCORES):
        idx_rel, dwin, key = per_core[c]
        cum = np.cumsum(counts[c])
        starts = cum - counts[c]
        rank = np.arange(len(key), dtype=np.int64) - starts[key]
        slot = grp_start_flat[key] * P + rank

        dwin_t = np.full((tc, P), -1.0, dtype=BF16)
        dwin_t[slot >> 7, slot & 127] = dwin.astype(BF16)
        dwin_cols = np.zeros((P, tc2), dtype=np.int16)
        dwin_cols[:, :tc] = np.ascontiguousarray(dwin_t.T).view(np.int16)

        idx_flat = np.zeros(tc * P, dtype=np.int16)
        idx_flat[slot] = idx_rel
        s_all = np.arange(tc * P, dtype=np.int64)
        idx_w = np.zeros((16, 8 * tc), dtype=np.int16)
        idx_w[s_all & 15, s_all >> 4] = idx_flat
        idx_full = np.tile(idx_w, (8, 1))

        node = c * shard + np.arange(n_win * P, dtype=np.int64)
        dsc = np.where(node < n, rs[np.minimum(node, n - 1)], np.float32(1.0))
        dsc = np.ascontiguousarray(
            dsc.reshape(n_win, P).T.astype(np.float32)
        ).view(np.int16)  # [P, 2*n_win]

        meta = np.concatenate([dwin_cols, iota_bits, dsc, idx_full], axis=1)
        assert meta.shape == (P, m16)
        tables.append(np.ascontiguousarray(meta))

    layout = dict(
        shard=shard,
        n_win=n_win,
        rows_pad=n_win * P,
        tc=tc,
        tc2=tc2,
        m16=m16,
        off_iota=off_iota,
        off_dsc=off_dsc,
        off_idx=off_idx,
        CS=CS,
        n_sw=n_sw,
        sw_base=sw_base,
        sw_chunks=sw_chunks,
        calls=calls,
        win_cols=win_cols,
    )
    return layout, tables, xs


def _trace_program(n, d, layout):
    from concourse import bacc, mybir
    import concourse.tile as tile

    f32 = mybir.dt.float32
    bf16 = mybir.dt.bfloat16
    i16 = mybir.dt.int16

    tc = layout["tc"]
    m16 = layout["m16"]
    CS = layout["CS"]
    n_sw = layout["n_sw"]
    n_win = layout["n_win"]
    sw_base = layout["sw_base"]
    sw_chunks = layout["sw_chunks"]
    calls = layout["calls"]
    win_cols = layout["win_cols"]

    calls_of_sw = [[] for _ in range(n_sw)]
    for b, c0, c1, s, reg in calls:
        calls_of_sw[s].append((b, c0, c1, reg))

    nc = bacc.Bacc(
        None, target_bir_lowering=False, debug=False, num_swdge_queues=NQ
    )
    xs_d = nc.declare_dram_parameter("xs", [n, d], bf16, isOutput=False)
    meta_d = nc.declare_dram_parameter("meta", [P, m16], i16, isOutput=False)
    y_d = nc.declare_dram_parameter(
        "y", [layout["rows_pad"], d], f32, isOutput=True
    )

    qrr = 0  # SWDGE queue round-robin

    with tile.TileContext(nc) as tc_ctx:
        with (
            tc_ctx.tile_pool(name="meta", bufs=1) as mpool,
            tc_ctx.tile_pool(name="gather", bufs=3) as gpool,
            tc_ctx.tile_pool(name="sel", bufs=2) as spool,
            tc_ctx.tile_pool(name="out", bufs=3) as opool,
            tc_ctx.tile_pool(name="acc", bufs=2, space="PSUM") as pspool,
        ):
            meta_sb = mpool.tile([P, m16], i16)
            nc.sync.dma_start(out=meta_sb[:], in_=meta_d[:])
            dwin_sb = meta_sb[:, 0:tc].bitcast(bf16)  # [P, tc]
            iota_sb = meta_sb[
                :, layout["off_iota"] : layout["off_iota"] + P
            ].bitcast(bf16)  # [P, P]
            dsc_sb = meta_sb[
                :, layout["off_dsc"] : layout["off_dsc"] + 2 * n_win
            ].bitcast(f32)  # [P, n_win]
            idx_sb = meta_sb[:, layout["off_idx"] :]  # [P, 8*tc] i16

            for s in range(n_sw):
                base = int(sw_base[s])
                swc = int(sw_chunks[s])
                g = gpool.tile([P, CS, d], bf16, tag="g")
                for b, c0, c1, reg in calls_of_sw[s]:
                    nc.gpsimd.dma_gather(
                        out_ap=g[:, c0 - base : c1 - base, :],
                        in_ap=xs_d[b * BK : min(n, (b + 1) * BK), :],
                        idxs_ap=idx_sb[:, c0 * 8 : c1 * 8],
                        num_idxs=(c1 - c0) * P,
                        num_idxs_reg=reg,
                        elem_size=d,
                        queue_num=qrr,
                    )
                    qrr = (qrr + 1) % NQ

                sel = spool.tile([P, CS, P], bf16, tag="s")
                nc.vector.tensor_tensor(
                    out=sel[:, 0:swc, :],
                    in0=iota_sb.unsqueeze(1).broadcast_to([P, swc, P]),
                    in1=dwin_sb[:, base : base + swc]
                    .unsqueeze(2)
                    .broadcast_to([P, swc, P]),
                    op=mybir.AluOpType.is_equal,
                )

                for w in range(s * SW, min((s + 1) * SW, n_win)):
                    cols = win_cols[w]
                    ps = pspool.tile([P, P], f32, tag="ps")
                    nq = len(cols)
                    for j in range(nq):
                        qc = cols[j] - base
                        nc.tensor.matmul(
                            out=ps[:],
                            lhsT=sel[:, qc, :],
                            rhs=g[:, qc, :],
                            start=(j == 0),
                            stop=(j == nq - 1),
                        )
                    o = opool.tile([P, P], f32, tag="o")
                    nc.scalar.activation(
                        out=o[:],
                        in_=ps[:],
                        func=mybir.ActivationFunctionType.Copy,
                        scale=dsc_sb[:, w : w + 1],
                    )
                    nc.sync.dma_start(
                        out=y_d[w * P : (w + 1) * P, :], in_=o[:]
                    )

    return nc


def _build_program(n, d, layout):
    nc = _trace_program(n, d, layout)
    nc.compile()
    return nc


def kernel(x, src, dst):
    x = np.ascontiguousarray(np.asarray(x, dtype=np.float32))
    src = np.asarray(src).astype(np.int64)
    dst = np.asarray(dst).astype(np.int64)
    n, d = x.shape

    layout, tables, xs = _plan(x, src, dst)

    key = (n, d, layout["tc"], tuple(layout["calls"]),
           tuple(tuple(q) for q in layout["win_cols"]))
    if key not in _CACHE:
        _CACHE[key] = _build_program(n, d, layout)
    nc = _CACHE[key]

    from concourse.bass_utils import run_bass_kernel_spmd

    in_maps = [{"xs": xs, "meta": tables[c]} for c in range(NCORES)]
    trace = os.environ.get("KERNEL_TRACE", "0") not in ("", "0")
    kw = {}
    if trace:
        kw["trace"] = True
        td = os.environ.get("KERNEL_TRACE_DIR")
        if td:
            kw["tmpdir"] = td
    res = run_bass_kernel_spmd(nc, in_maps, list(range(NCORES)), **kw)
    global LAST_RESULT
    LAST_RESULT = res

    shard = layout["shard"]
    out = np.empty((n, d), dtype=np.float32)
    for c in range(NCORES):
        lo = c * shard
        hi = min(n, lo + shard)
        out[lo:hi] = res.results[c]["y"][: hi - lo]
    return out
